# revision 1
# baseline (speedup 1.0000x reference)
"""Trainium2 Bass kernel for nn_DinoGazeSpade (segment_reduce + repaint).

reference semantics:
  seg_feat = mask[:, ::14, ::14]                       # nearest-downsample to 28x28
  seg_avg[b, s, :] = mean of feat pixels with seg==s   # scatter_mean over B*128 segments
  out[b, :, hi, wi] = seg_avg[b, mask[b, hi, wi], :]   # repaint at full res

Sharding: 8 cores = 2 batches x 4 row-slices of the 392-row full-res output.
Each core computes its batch's seg_avg table (tiny) and paints its 98-row
slice. The paint is a one-hot(segment) x seg_avg matmul on the tensor engine,
which directly produces the channel-major output layout. seg_avg is split
hi/lo into two bf16 matrices so two bf16 matmuls reproduce fp32 accuracy.
"""

import numpy as np
from contextlib import ExitStack

import concourse.bass as bass
import concourse.tile as tile
from concourse import bacc, mybir
from concourse.bass_utils import run_bass_kernel_spmd

# problem shape (hardcoded per contract)
B, C, Hp, Wp = 2, 768, 28, 28
Hi, Wi = 392, 392
S = 128                    # segments per image
N_CORES = 8
ROWS = Hi // 4             # 98 full-res rows per core
NPIX = ROWS * Wi           # 38416 pixels per core
NPATCH = Hp * Wp           # 784 patch pixels
PCHUNK = 112               # 784 = 7 * 112 patch-pixel chunks (partition dim)
PTILE = 512                # paint pixel tile (one PSUM bank)
GROUP = 3 * PTILE          # 1536 pixels per paint group
NGROUP = NPIX // GROUP     # 25 full groups
REM = NPIX - NGROUP * GROUP  # 16 remainder pixels
CT = C // 128              # 6 channel tiles

f32 = mybir.dt.float32
bf16 = mybir.dt.bfloat16
i32 = mybir.dt.int32

_CACHED_NC = None


def _build_nc():
    nc = bacc.Bacc()
    feat_hbm = nc.dram_tensor("feat", [NPATCH, C], f32, kind="ExternalInput")
    pmask_hbm = nc.dram_tensor("pmask", [NPATCH], f32, kind="ExternalInput")
    mask_hbm = nc.dram_tensor("mask", [1, NPIX], f32, kind="ExternalInput")
    out_hbm = nc.dram_tensor("out", [C, NPIX], f32, kind="ExternalOutput")

    with tile.TileContext(nc) as tc, ExitStack() as ctx:
        const = ctx.enter_context(tc.tile_pool(name="const", bufs=1))
        segp = ctx.enter_context(tc.tile_pool(name="segp", bufs=1))

        # ---- constants ----
        iota_pi = const.tile([128, 1], i32)           # partition index
        nc.gpsimd.iota(iota_pi[:], [[0, 1]], channel_multiplier=1)
        iota_pf = const.tile([128, 1], f32)
        nc.vector.tensor_copy(iota_pf[:], iota_pi[:])
        iota_ri = const.tile([128, 128], i32)         # free-dim index (same per partition)
        nc.gpsimd.iota(iota_ri[:], [[1, 128]], channel_multiplier=0)
        iota_rf = const.tile([128, 128], f32)
        nc.vector.tensor_copy(iota_rf[:], iota_ri[:])
        ones_bf = const.tile([1, 128], bf16)
        nc.vector.memset(ones_bf[:], 1.0)
        ones_f = const.tile([128, 1], f32)
        nc.vector.memset(ones_f[:], 1.0)

        # ---- phase A: scatter-mean over patch pixels -> seg_avg [S=128, C] ----
        seg_sb = segp.tile([128, C], f32)
        hi_bf = segp.tile([128, C], bf16)
        lo_bf = segp.tile([128, C], bf16)

        with tc.tile_pool(name="psA", bufs=1, space="PSUM") as psA, \
             tc.tile_pool(name="sbA", bufs=2) as sbA:
            sums0 = psA.tile([128, 384], f32)
            sums1 = psA.tile([128, 384], f32)
            cnt_ps = psA.tile([128, 1], f32)
            for k in range(NPATCH // PCHUNK):
                pm = sbA.tile([PCHUNK, 1], f32, tag="pm")
                nc.sync.dma_start(
                    out=pm[:], in_=pmask_hbm[k * PCHUNK:(k + 1) * PCHUNK, None]
                )
                oh = sbA.tile([PCHUNK, 128], f32, tag="ohp")
                nc.vector.tensor_tensor(
                    out=oh[:], in0=iota_rf[0:PCHUNK, :],
                    in1=pm[:].to_broadcast([PCHUNK, 128]),
                    op=mybir.AluOpType.is_equal,
                )
                fch = sbA.tile([PCHUNK, C], f32, tag="fch")
                nc.sync.dma_start(
                    out=fch[:], in_=feat_hbm[k * PCHUNK:(k + 1) * PCHUNK, :]
                )
                first, last = k == 0, k == NPATCH // PCHUNK - 1
                nc.tensor.matmul(sums0[:], lhsT=oh[:], rhs=fch[:, 0:384],
                                 start=first, stop=last)
                nc.tensor.matmul(sums1[:], lhsT=oh[:], rhs=fch[:, 384:768],
                                 start=first, stop=last)
                nc.tensor.matmul(cnt_ps[:], lhsT=oh[:], rhs=ones_f[0:PCHUNK, :],
                                 start=first, stop=last)

            # r = 1 / max(cnt, 1); empty segments have sums == 0 so avg == 0
            cnt_sb = sbA.tile([128, 1], f32)
            nc.vector.tensor_scalar_max(cnt_sb[:], cnt_ps[:], 1.0)
            rcp = sbA.tile([128, 1], f32)
            nc.vector.reciprocal(rcp[:], cnt_sb[:])
            nc.vector.tensor_scalar(
                out=seg_sb[:, 0:384], in0=sums0[:], scalar1=rcp[:], scalar2=None,
                op0=mybir.AluOpType.mult,
            )
            nc.vector.tensor_scalar(
                out=seg_sb[:, 384:768], in0=sums1[:], scalar1=rcp[:], scalar2=None,
                op0=mybir.AluOpType.mult,
            )
            # hi/lo bf16 split: seg = hi + lo to ~fp32 accuracy
            nc.vector.tensor_copy(hi_bf[:], seg_sb[:])
            hi_f = sbA.tile([128, C], f32)
            nc.vector.tensor_copy(hi_f[:], hi_bf[:])
            lo_f = sbA.tile([128, C], f32)
            nc.vector.tensor_sub(lo_f[:], seg_sb[:], hi_f[:])
            nc.vector.tensor_copy(lo_bf[:], lo_f[:])

        # ---- phase B: paint full-res pixels ----
        with tc.tile_pool(name="psB", bufs=2, space="PSUM") as psB, \
             tc.tile_pool(name="psO", bufs=6, space="PSUM") as psO, \
             tc.tile_pool(name="sbB", bufs=3) as sbB, \
             tc.tile_pool(name="osb", bufs=8) as osb:

            def paint(pix0, npx, nt, ptile):
                # one group: pixels [pix0, pix0+npx), nt tiles of ptile pixels
                mch_f = sbB.tile([1, npx], f32, tag="mchf")
                nc.sync.dma_start(out=mch_f[:], in_=mask_hbm[0:1, pix0:pix0 + npx])
                mch_bf = sbB.tile([1, npx], bf16, tag="mchb")
                nc.vector.tensor_copy(mch_bf[:], mch_f[:])
                ohs = []
                for t in range(nt):
                    bc = psB.tile([128, ptile], f32, tag="bc")
                    nc.tensor.matmul(
                        bc[:], lhsT=ones_bf[:],
                        rhs=mch_bf[0:1, t * ptile:(t + 1) * ptile],
                        start=True, stop=True,
                    )
                    oh = sbB.tile([128, ptile], bf16, tag="ohb")
                    nc.vector.tensor_scalar(
                        out=oh[:], in0=bc[:], scalar1=iota_pf[:], scalar2=None,
                        op0=mybir.AluOpType.is_equal,
                    )
                    ohs.append(oh)
                for c in range(CT):
                    ob = osb.tile([128, npx], f32, tag="ob")
                    for t in range(nt):
                        op = psO.tile([128, ptile], f32, tag="op")
                        nc.tensor.matmul(op[:], lhsT=hi_bf[:, c * 128:(c + 1) * 128],
                                         rhs=ohs[t][:], start=True, stop=False)
                        nc.tensor.matmul(op[:], lhsT=lo_bf[:, c * 128:(c + 1) * 128],
                                         rhs=ohs[t][:], start=False, stop=True)
                        nc.vector.tensor_copy(ob[:, t * ptile:(t + 1) * ptile], op[:])
                    nc.sync.dma_start(
                        out=out_hbm[c * 128:(c + 1) * 128, pix0:pix0 + npx], in_=ob[:]
                    )

            for g in range(NGROUP):
                paint(g * GROUP, GROUP, 3, PTILE)
            if REM:
                paint(NGROUP * GROUP, REM, 1, REM)

    nc.compile()
    return nc


def kernel(F_semantic_patches: np.ndarray, segmentation_mask: np.ndarray) -> np.ndarray:
    global _CACHED_NC
    if _CACHED_NC is None:
        _CACHED_NC = _build_nc()
    nc = _CACHED_NC

    F = np.asarray(F_semantic_patches, dtype=np.float32)
    M = np.asarray(segmentation_mask)

    in_maps = []
    for core in range(N_CORES):
        b, q = divmod(core, 4)
        feat = np.ascontiguousarray(F[b].reshape(C, NPATCH).T)        # [784, 768]
        pmask = np.ascontiguousarray(
            M[b, ::Hi // Hp, ::Wi // Wp].reshape(NPATCH)
        ).astype(np.float32)
        mask = np.ascontiguousarray(
            M[b, q * ROWS:(q + 1) * ROWS, :].reshape(1, NPIX)
        ).astype(np.float32)
        in_maps.append({"feat": feat, "pmask": pmask, "mask": mask})

    res = run_bass_kernel_spmd(nc, in_maps, core_ids=list(range(N_CORES)))

    out = np.empty((B, C, Hi, Wi), dtype=np.float32)
    for core in range(N_CORES):
        b, q = divmod(core, 4)
        out[b, :, q * ROWS:(q + 1) * ROWS, :] = (
            res.results[core]["out"].reshape(C, ROWS, Wi)
        )
    return out
